# revision 19
# baseline (speedup 1.0000x reference)
"""Trainium2 Bass kernel for nn_CTNet (conv -> BN -> MHSA -> BN -> maxpool -> MLP).

Data-parallel over 8 NeuronCores: batch 4096 -> 512/core (padded to 516 =
43 attention rounds x 12 samples). Training-mode BatchNorm batch stats are
made global with two tiny AllReduces ([128,8] mean/E[x^2] payloads).

Layouts (per core):
  xf / q / k / av : [C(4x128 part), pos]   pos = sample*10 + s
  vT              : [pos(120-chunks), C]   (direct transposed projection)
  attention       : 12 samples/round; 4 PE column-strips x 3 samples each,
                    3-sample Gram blocks masked to the diagonal, softmax over
                    30 cols with -1e30 off-block mask, attn^T via PE
                    transpose into a 120x120 block-diagonal operand
  pooled          : [C, (p 8, b 516)] ; maxpool commutes with the (positive
                    scale) BN2 affine + ReLU, which are applied post-pool
  FC1             : weights host-permuted to [(p*512+c), h]; rhs = pooled
                    slices; out [h, b]; 2 PSUM waves of 8 h-chunks
"""
import sys
import functools
from contextlib import ExitStack

sys.path.insert(0, "/opt/trn_rl_repo")

import numpy as np
import concourse.bass as bass  # noqa: E402
import concourse.tile as tile  # noqa: E402
from concourse import bacc, mybir  # noqa: E402
from concourse.bass_utils import run_bass_kernel_spmd  # noqa: E402

N_CORES = 8
B = 4096
B_LOC = 512
B_PAD = 516
S = 10
C = 512
CH, H, W = 7, 4, 17
KH, KW = 4, 10
KFEAT = 280
POS_LOC = B_PAD * S        # 5160
POS_REAL = B_LOC * S       # 5120
HID = 2048
EPS = 1e-5
SB_SAMP = 36
N_SB = 15                  # 14 x 36 + 12 samples
NEG = -1e30

F32 = mybir.dt.float32
F32R = mybir.dt.float32r
AF = mybir.ActivationFunctionType
ALU = mybir.AluOpType
AX = mybir.AxisListType





def mask30_np():
    m = np.full((128, 30), NEG, np.float32)
    for c in range(4):
        for s in range(3):
            m[32 * c + 10 * s: 32 * c + 10 * s + 10, 10 * s: 10 * s + 10] = 0.0
    return m


def identb_np():
    return np.eye(128, dtype=np.float32)


def _emit(nc, tc, ctx, d):
    const = ctx.enter_context(tc.tile_pool(name="const", bufs=1))
    small = ctx.enter_context(tc.tile_pool(name="small", bufs=4))

    def load_const(dram, shape, name=None, dt=F32):
        t = const.tile(list(shape), dt, tag=name, name=name)
        if dram.dtype != dt:
            dram = dram.bitcast(dt)
        nc.sync.dma_start(t[:, :], dram)
        return t

    # ---- resident parameters ----
    wim_t = [load_const(d["wim"][k0:k1, :], [k1 - k0, 512], f"wim{i}", F32R)
             for i, (k0, k1) in enumerate(((0, 128), (128, 256), (256, 280)))]
    qT_t = [load_const(d["qT"][kc * 128:(kc + 1) * 128, :], [128, 512], f"qT{kc}", F32R)
            for kc in range(4)]
    kT_t = [load_const(d["kT"][kc * 128:(kc + 1) * 128, :], [128, 512], f"kT{kc}", F32R)
            for kc in range(4)]
    vwT_t = [load_const(d["vwT"][kc * 128:(kc + 1) * 128, :], [128, 512], f"vwT{kc}", F32R)
             for kc in range(4)]
    pos3_t = [load_const(d["pos3"][kc * 128:(kc + 1) * 128, :], [128, 30], f"pos3{kc}", F32R)
              for kc in range(4)]
    fc2T_t = [load_const(d["fc2T"][h8 * 128:(h8 + 1) * 128, :], [128, 2], f"fc2T{h8}", F32R)
              for h8 in range(16)]

    def vec4(name):
        return [load_const(d[name][mc * 128:(mc + 1) * 128, :], [128, 1],
                           f"{name}{mc}") for mc in range(4)]

    cb_t = vec4("conv_b")
    qb_t = vec4("q_b")
    kb_t = vec4("k_b")
    vb_t = vec4("v_b")
    bn1g_t = vec4("bn1_g")
    bn1b_t = vec4("bn1_b")
    bn2g_t = vec4("bn2_g")
    bn2b_t = vec4("bn2_b")
    fc1b_t = [load_const(d["fc1_b"][h8 * 128:(h8 + 1) * 128, :], [128, 1],
                         f"fc1b{h8}") for h8 in range(16)]
    fc2b_t = load_const(d["fc2_b"], [2, 1], "fc2b")
    mask_t = load_const(d["mask30"].ap(), [128, 30], "mask")
    id128_t = load_const(d["identb"].ap(), [128, 128], "identb", F32R)

    # ---- persistent state ----
    scores = const.tile([128, 43 * 30], F32, tag="scores", name="scores")
    bd = const.tile([128, 256], F32R, tag="bd", name="bd")
    nc.sync.dma_start(bd[:, :], d["zeros"].ap().bitcast(F32R))
    stats1 = [const.tile([128, 72], F32, tag=f"st1_{mc}", name=f"st1_{mc}") for mc in range(4)]
    stats2 = [const.tile([128, 264], F32, tag=f"st2_{mc}", name=f"st2_{mc}") for mc in range(4)]
    pooled = [const.tile([128, 8 * B_PAD], F32R, tag=f"pool{mc}", name=f"pool{mc}") for mc in range(4)]
    scale1 = [const.tile([128, 1], F32, tag=f"sc1_{mc}", name=f"sc1_{mc}") for mc in range(4)]
    shift1 = [const.tile([128, 1], F32, tag=f"sh1_{mc}", name=f"sh1_{mc}") for mc in range(4)]
    scale2 = [const.tile([128, 1], F32, tag=f"sc2_{mc}", name=f"sc2_{mc}") for mc in range(4)]
    shift2 = [const.tile([128, 1], F32, tag=f"sh2_{mc}", name=f"sh2_{mc}") for mc in range(4)]

    # ================= PASS 1: conv =================
    with (
        tc.tile_pool(name="im2p", bufs=6) as im2p,
        tc.tile_pool(name="cvps", bufs=4, space="PSUM") as cvps,
        tc.tile_pool(name="xfst", bufs=4) as xfst,
    ):
        for nb in range(12):
            n0 = nb * 430
            im2t = []
            for i, (k0, k1) in enumerate(((0, 128), (128, 256), (256, 280))):
                t = im2p.tile([128, 430], F32R, tag="im2t", name="im2t")
                nc.sync.dma_start(t[: k1 - k0, :], d["im2"][k0:k1, n0:n0 + 430])
                im2t.append(t)
            for mc in range(4):
                ps = cvps.tile([128, 430], F32, name="cvps")
                for i, (k0, k1) in enumerate(((0, 128), (128, 256), (256, 280))):
                    nc.tensor.matmul(
                        ps[:, :],
                        wim_t[i][: k1 - k0, mc * 128:(mc + 1) * 128],
                        im2t[i][: k1 - k0, :],
                        start=(i == 0), stop=(i == 2))
                xs = xfst.tile([128, 430], F32)
                nc.scalar.activation(xs[:, :], ps[:, :], AF.Identity, bias=cb_t[mc][:, :])
                w = 430 if nb < 11 else 390
                nc.vector.bn_stats(stats1[mc][:, nb * 6:(nb + 1) * 6], xs[:, :w])
                nc.sync.dma_start(
                    d["xf"][mc * 128:(mc + 1) * 128, n0:n0 + 430], xs[:, :])

    # ================= AllReduce 1 (BN1 stats) =================
    def bn_allreduce(statsbufs, cc_in, cc_out, g_t, b_t, scale_t, shift_t,
                     corr=None):
        # corr = (a, b, dead_t): true mean = a*mean_meas - b*dead,
        #        true E[x^2] = a*e2_meas - b*dead^2
        pay = small.tile([128, 8], F32, tag="pay", name="pay")
        eps_t = small.tile([128, 1], F32, tag="eps", name="eps")
        nc.vector.memset(eps_t[:, :], EPS)
        for mc in range(4):
            loc = small.tile([128, 2], F32, tag="bnloc", name="bnloc")
            nc.vector.bn_aggr(loc[:, :], statsbufs[mc][:, :])
            e2m = small.tile([128, 1], F32, tag="e2m", name="e2m")
            nc.vector.scalar_tensor_tensor(
                e2m[:, :], loc[:, 0:1], loc[:, 0:1],
                loc[:, 1:2], op0=ALU.mult, op1=ALU.add)
            if corr is None:
                nc.vector.tensor_copy(pay[:, 2 * mc:2 * mc + 1], loc[:, 0:1])
                nc.vector.tensor_copy(pay[:, 2 * mc + 1:2 * mc + 2], e2m[:, :])
            else:
                ca, cb, dead_t = corr
                db = small.tile([128, 1], F32, tag="db", name="db")
                nc.scalar.mul(db[:, :], dead_t[mc][:, :], cb)
                nc.vector.scalar_tensor_tensor(
                    pay[:, 2 * mc:2 * mc + 1], loc[:, 0:1], ca, db[:, :],
                    op0=ALU.mult, op1=ALU.subtract)
                d2b = small.tile([128, 1], F32, tag="d2b", name="d2b")
                nc.vector.tensor_mul(d2b[:, :], db[:, :], dead_t[mc][:, :])
                nc.vector.scalar_tensor_tensor(
                    pay[:, 2 * mc + 1:2 * mc + 2], e2m[:, :], ca, d2b[:, :],
                    op0=ALU.mult, op1=ALU.subtract)
        nc.sync.dma_start(cc_in.ap(), pay[:, :])
        nc.gpsimd.collective_compute(
            "AllReduce", ALU.add, replica_groups=[list(range(N_CORES))],
            ins=[cc_in.ap()], outs=[cc_out.ap()])
        ar = small.tile([128, 8], F32, tag="ar", name="ar")
        nc.sync.dma_start(ar[:, :], cc_out.ap())
        for mc in range(4):
            mg = small.tile([128, 1], F32, tag="mg", name="mg")
            e2 = small.tile([128, 1], F32, tag="e2", name="e2")
            nc.scalar.mul(mg[:, :], ar[:, 2 * mc:2 * mc + 1], 1.0 / N_CORES)
            nc.scalar.mul(e2[:, :], ar[:, 2 * mc + 1:2 * mc + 2], 1.0 / N_CORES)
            mg2 = small.tile([128, 1], F32, tag="mg2", name="mg2")
            nc.vector.tensor_mul(mg2[:, :], mg[:, :], mg[:, :])
            vg = small.tile([128, 1], F32, tag="vg", name="vg")
            nc.vector.tensor_sub(vg[:, :], e2[:, :], mg2[:, :])
            sq = small.tile([128, 1], F32, tag="sq", name="sq")
            nc.scalar.activation(sq[:, :], vg[:, :], AF.Sqrt, bias=eps_t[:, :])
            rstd = small.tile([128, 1], F32, tag="rstd", name="rstd")
            nc.vector.reciprocal(rstd[:, :], sq[:, :])
            nc.vector.tensor_mul(scale_t[mc][:, :], rstd[:, :], g_t[mc][:, :])
            ms = small.tile([128, 1], F32, tag="ms", name="ms")
            nc.vector.tensor_mul(ms[:, :], mg[:, :], scale_t[mc][:, :])
            nc.vector.tensor_sub(shift_t[mc][:, :], b_t[mc][:, :], ms[:, :])

    bn_allreduce(stats1, d["cc1i"], d["cc1o"], bn1g_t, bn1b_t, scale1, shift1)

    # ================= PASS 2: attention super-blocks =================
    with (
        tc.tile_pool(name="xfl", bufs=4) as xfl,
        tc.tile_pool(name="xfn", bufs=8) as xfnp,
        tc.tile_pool(name="qk", bufs=16) as qkp,
        tc.tile_pool(name="vt", bufs=6) as vtp,
        tc.tile_pool(name="att", bufs=4) as attp,
        tc.tile_pool(name="avs", bufs=8) as avsp,
        tc.tile_pool(name="ptmp", bufs=4) as ptmpp,
        tc.tile_pool(name="qkps", bufs=2, space="PSUM") as qkps,
        tc.tile_pool(name="vtps", bufs=2, space="PSUM") as vtps,
        tc.tile_pool(name="scps", bufs=1, space="PSUM") as scps,
        tc.tile_pool(name="trps", bufs=1, space="PSUM") as trps,
        tc.tile_pool(name="avps", bufs=2, space="PSUM") as avps,
    ):
        for sb in range(N_SB):
            nsamp = SB_SAMP if sb < 14 else 12
            L = nsamp * S
            n_r = nsamp // 12
            LG = n_r * 128          # gapped width: rounds are 4 strips x 32
            pos0 = sb * SB_SAMP * S
            glob0 = sb * SB_SAMP
            # load + BN1-normalize xf (dense DRAM -> gapped SBUF layout)
            xfn = []
            for mc in range(4):
                xr = xfl.tile([128, 3 * 128], F32, tag="xfl", name="xfl")
                nc.vector.memset(xr[:, :LG], 0.0)
                src = d["xf"][mc * 128:(mc + 1) * 128, pos0:pos0 + L].rearrange(
                    "c (r t k) -> c r t k", r=n_r, t=4)
                dst = xr[:, :LG].rearrange("c (r t k) -> c r t k",
                                           r=n_r, t=4, k=32)[:, :, :, 0:30]
                nc.sync.dma_start(dst, src)
                xn = xfnp.tile([128, 3 * 128], F32R, tag="xfn", name="xfn")
                nc.scalar.activation(xn[:, :LG], xr[:, :LG], AF.Relu,
                                     bias=shift1[mc][:, :], scale=scale1[mc][:, :])
                xfn.append(xn)
            # q, k projections [C, LG]
            q_t, k_t = [], []
            for wT, bias_t, outlist in ((qT_t, qb_t, q_t), (kT_t, kb_t, k_t)):
                for mc in range(4):
                    ps = qkps.tile([128, 3 * 128], F32, name="qkps")
                    for kc in range(4):
                        nc.tensor.matmul(
                            ps[:, :LG],
                            wT[kc][:, mc * 128:(mc + 1) * 128],
                            xfn[kc][:, :LG],
                            start=(kc == 0), stop=(kc == 3))
                    o = qkp.tile([128, 3 * 128], F32R, tag="qk", name="qk")
                    nc.scalar.activation(o[:, :LG], ps[:, :LG], AF.Identity,
                                         bias=bias_t[mc][:, :])
                    outlist.append(o)
            # vT [128-chunk (gapped pos), C]
            vt_t = []
            for pc in range(n_r):
                ps = vtps.tile([128, 512], F32, name="vtps")
                for kc in range(4):
                    nc.tensor.matmul(
                        ps[:, :],
                        xfn[kc][:, pc * 128:(pc + 1) * 128],
                        vwT_t[kc][:, :],
                        start=(kc == 0), stop=(kc == 3))
                vt = vtp.tile([128, 512], F32R, tag="vt", name="vt")
                nc.vector.tensor_copy(vt[:, :], ps[:, :])
                vt_t.append(vt)
            # scores per round
            for r in range(n_r):
                g = sb * 3 + r
                ps = scps.tile([128, 30], F32, name="scps")
                for strip in range(4):
                    sl0 = r * 128 + strip * 32
                    mm = 0
                    for kc in range(4):
                        nc.tensor.matmul(
                            ps[32 * strip:32 * strip + 30, :],
                            q_t[kc][:, sl0:sl0 + 30].bitcast(F32),
                            k_t[kc][:, sl0:sl0 + 30].bitcast(F32),
                            start=(mm == 0), stop=False,
                            tile_position=(0, 32 * strip))
                        mm += 1
                    for kc in range(4):
                        nc.tensor.matmul(
                            ps[32 * strip:32 * strip + 30, :],
                            pos3_t[kc][:, :].bitcast(F32),
                            q_t[kc][:, sl0:sl0 + 30].bitcast(F32),
                            start=False, stop=(mm == 7),
                            tile_position=(0, 32 * strip))
                        mm += 1
                nc.vector.scalar_tensor_tensor(
                    scores[:, g * 30:(g + 1) * 30], ps[:, :], 1.0, mask_t[:, :],
                    op0=ALU.mult, op1=ALU.add)
            # softmax over this SB's rounds
            g0 = sb * 3
            sc3 = scores[:, g0 * 30:(g0 + n_r) * 30].rearrange(
                "p (r j) -> p r j", r=n_r)
            mx = small.tile([128, 4], F32, tag="mx", name="mx")
            nc.vector.tensor_reduce(mx[:, :n_r], sc3, axis=AX.X, op=ALU.max)
            att = attp.tile([128, 90], F32, tag="att", name="att")
            att3 = att[:, :n_r * 30].rearrange("p (r j) -> p r j", r=n_r)
            mx3 = mx[:, :n_r].unsqueeze(2).broadcast_to([128, n_r, 30])
            nc.vector.tensor_tensor(att3, sc3, mx3, op=ALU.subtract)
            nc.scalar.activation(att[:, :n_r * 30], att[:, :n_r * 30], AF.Exp)
            sm = small.tile([128, 4], F32, tag="sm", name="sm")
            nc.vector.tensor_reduce(sm[:, :n_r], att3, axis=AX.X, op=ALU.add)
            rs = small.tile([128, 4], F32, tag="rs", name="rs")
            nc.vector.reciprocal(rs[:, :n_r], sm[:, :n_r])
            attn = attp.tile([128, 90], F32R, tag="attn", name="attn")
            attn3 = attn[:, :n_r * 30].rearrange("p (r j) -> p r j", r=n_r)
            rs3 = rs[:, :n_r].unsqueeze(2).broadcast_to([128, n_r, 30])
            nc.vector.tensor_tensor(attn3, att3, rs3, op=ALU.mult)
            # av per round
            av_t = []
            for mc in range(4):
                av_t.append(avsp.tile([128, 3 * 128], F32, tag="avs", name="avs"))
            for r in range(n_r):
                g = sb * 3 + r
                half = g % 2
                pst = trps.tile([30, 128], F32R, name="trps")
                nc.tensor.transpose(
                    pst[:, :],
                    attn[:, r * 30:(r + 1) * 30],
                    id128_t[:, :])
                for strip in range(4):
                    nc.scalar.copy(
                        bd[32 * strip:32 * strip + 30,
                           half * 128 + 32 * strip: half * 128 + 32 * strip + 30],
                        pst[0:30, 32 * strip:32 * strip + 30])
                for mc in range(4):
                    ap = avps.tile([128, 128], F32, name="avpst")
                    nc.tensor.matmul(
                        ap[:, :],
                        vt_t[r][:, mc * 128:(mc + 1) * 128],
                        bd[:, half * 128: half * 128 + 128],
                        start=True, stop=True)
                    nc.scalar.activation(
                        av_t[mc][:, r * 128:(r + 1) * 128], ap[:, :],
                        AF.Identity, bias=vb_t[mc][:, :])
            # BN2 stats: whole 128-col round blocks; the 8 dead lanes per
            # block hold exactly v_b (bd zeros kill everything else) and are
            # corrected analytically in bn_allreduce. Round 42: only strips
            # 0,1 (60 real + 4 dead) and the 20 real cols of strip 2.
            for r in range(n_r):
                g = sb * 3 + r
                for mc in range(4):
                    avr = av_t[mc][:, r * 128:(r + 1) * 128]
                    if g < 42:
                        nc.vector.bn_stats(
                            stats2[mc][:, g * 6:(g + 1) * 6], avr[:, :])
                    else:
                        nc.vector.bn_stats(
                            stats2[mc][:, g * 6:(g + 1) * 6], avr[:, 0:64])
                        nc.vector.bn_stats(
                            stats2[mc][:, (g + 1) * 6:(g + 2) * 6],
                            avr[:, 64:84])
            # maxpool (1,3) -> pooled [C, (p 8, b 516)]
            for mc in range(4):
                po4 = pooled[mc].rearrange("c (p g u) -> c g u p",
                                           p=8, g=43, u=12)
                for strip in range(4):
                    av5 = av_t[mc][:, :LG].rearrange(
                        "c (r t k) -> c r t k", r=n_r, t=4, k=32)[
                            :, :, strip, 0:30].rearrange(
                            "c r (s i) -> c r s i", s=3)
                    pt = ptmpp.tile([128, 3 * 24], F32, tag="ptmp", name="ptmp")
                    pt4 = pt[:, :n_r * 24].rearrange(
                        "c (r s p) -> c r s p", r=n_r, s=3)
                    nc.vector.tensor_tensor(pt4, av5[:, :, :, 0:8],
                                            av5[:, :, :, 1:9], op=ALU.max)
                    nc.vector.tensor_tensor(
                        po4[:, sb * 3:sb * 3 + n_r,
                            3 * strip:3 * strip + 3, :],
                        pt4, av5[:, :, :, 2:10], op=ALU.max)

    # ================= AllReduce 2 (BN2 stats) =================
    n_meas = 42 * 128 + 64 + 20
    bn_allreduce(stats2, d["cc2i"], d["cc2o"], bn2g_t, bn2b_t, scale2, shift2,
                 corr=(n_meas / POS_REAL, (n_meas - POS_REAL) / POS_REAL, vb_t))

    # ================= PASS 3: BN2 affine + FC head =================
    for mc in range(4):
        nc.scalar.activation(pooled[mc][:, :], pooled[mc][:, :], AF.Relu,
                             bias=shift2[mc][:, :], scale=scale2[mc][:, :])
    with tc.tile_pool(name="h1p", bufs=1) as h1p:
      h1_t = [h1p.tile([128, B_LOC], F32R, tag=f"h1_{h8}", name=f"h1_{h8}")
              for h8 in range(16)]
      with (
        tc.tile_pool(name="fc1w", bufs=3) as fc1wp,
        tc.tile_pool(name="fc1ps", bufs=8, space="PSUM") as fc1ps,
      ):
        for wave in range(2):
            hps = [fc1ps.tile([128, 512], F32, name="fc1pst") for _ in range(8)]
            for fk in range(32):
                p, mc = divmod(fk, 4)
                fkt = fc1wp.tile([128, 2048], F32R, tag="fc1w", name="fc1w")
                nc.sync.dma_start(fkt[:, :], d["fc1p"][fk * 128:(fk + 1) * 128, :])
                for i in range(8):
                    h8 = wave * 8 + i
                    nc.tensor.matmul(
                        hps[i][:, :],
                        fkt[:, h8 * 128:(h8 + 1) * 128],
                        pooled[mc][:, p * B_PAD: p * B_PAD + B_LOC],
                        start=(fk == 0), stop=(fk == 31))
            for i in range(8):
                h8 = wave * 8 + i
                nc.scalar.activation(h1_t[h8][:, :], hps[i][:, :], AF.Relu,
                                     bias=fc1b_t[h8][:, :])
      with (
          tc.tile_pool(name="fc2ps", bufs=1, space="PSUM") as fc2ps,
          tc.tile_pool(name="outp", bufs=1) as outp,
      ):
        ps = fc2ps.tile([2, 512], F32, name="fc2pst")
        for h8 in range(16):
            nc.tensor.matmul(ps[:, :], fc2T_t[h8][:, :], h1_t[h8][:, :],
                             start=(h8 == 0), stop=(h8 == 15))
        out_sb = outp.tile([2, 512], F32, tag="outsb", name="outsb")
        nc.scalar.activation(out_sb[:, :], ps[:, :], AF.Sigmoid,
                             bias=fc2b_t[:, :])
        nc.sync.dma_start(d["out"].ap(), out_sb[:, :])


@functools.lru_cache(maxsize=1)
def build():
    nc = bacc.Bacc("TRN2", target_bir_lowering=False, debug=False,
                   num_devices=N_CORES)
    d = {}
    d["im2"] = nc.dram_tensor("im2", [KFEAT, POS_LOC], F32R,
                              kind="ExternalInput").ap()
    for name, shape in (
        ("wim", [KFEAT, 512]), ("qT", [512, 512]), ("kT", [512, 512]),
        ("vwT", [512, 512]), ("pos3", [512, 30]), ("conv_b", [512, 1]),
        ("q_b", [512, 1]), ("k_b", [512, 1]), ("v_b", [512, 1]),
        ("bn1_g", [512, 1]), ("bn1_b", [512, 1]), ("bn2_g", [512, 1]),
        ("bn2_b", [512, 1]), ("fc1p", [C * 8, HID]), ("fc1_b", [HID, 1]),
        ("fc2T", [HID, 2]), ("fc2_b", [2, 1]),
    ):
        dt = F32R if name in ("wim", "qT", "kT", "vwT", "pos3", "fc1p",
                              "fc2T") else F32
        d[name] = nc.dram_tensor(name, shape, dt, kind="ExternalInput").ap()
    d["out"] = nc.dram_tensor("out", [2, B_LOC], F32, kind="ExternalOutput")
    d["xf"] = nc.dram_tensor("xf", [C, POS_LOC], F32).ap()
    d["cc1i"] = nc.dram_tensor("cc1i", [128, 8], F32)
    d["cc1o"] = nc.dram_tensor("cc1o", [128, 8], F32, addr_space="Shared")
    d["cc2i"] = nc.dram_tensor("cc2i", [128, 8], F32)
    d["cc2o"] = nc.dram_tensor("cc2o", [128, 8], F32, addr_space="Shared")
    d["mask30"] = nc.inline_tensor(mask30_np(), name="mask30")
    d["identb"] = nc.inline_tensor(identb_np(), name="identb")
    d["zeros"] = nc.inline_tensor(np.zeros((128, 256), np.float32), name="zeros")

    with tile.TileContext(nc) as tc:
        with ExitStack() as ctx:
            _emit(nc, tc, ctx, d)
    nc.compile()
    return nc


def host_prep(inp):
    x = np.asarray(inp["x"], np.float32)
    x_pad = np.zeros((B, CH, H, W + 2), np.float32)
    x_pad[:, :, :, 1:-1] = x
    sw = np.lib.stride_tricks.sliding_window_view(x_pad, KW, axis=3)
    im2colT = np.ascontiguousarray(
        sw.transpose(1, 2, 4, 0, 3).reshape(KFEAT, B * S))
    prep = {}
    prep["wim"] = np.ascontiguousarray(
        np.asarray(inp["conv_w"], np.float32).reshape(C, KFEAT).T)
    prep["qT"] = np.ascontiguousarray(np.asarray(inp["q_w"], np.float32).T)
    prep["kT"] = np.ascontiguousarray(np.asarray(inp["k_w"], np.float32).T)
    prep["vwT"] = np.ascontiguousarray(np.asarray(inp["v_w"], np.float32).T)
    pos = (np.asarray(inp["rel_h"], np.float32)
           + np.asarray(inp["rel_w"], np.float32)).reshape(C, S)
    prep["pos3"] = np.ascontiguousarray(np.concatenate([pos] * 3, axis=1))
    prep["fc1p"] = np.ascontiguousarray(
        np.asarray(inp["fc1_w"], np.float32).reshape(HID, C, 8)
        .transpose(2, 1, 0).reshape(C * 8, HID))
    prep["fc2T"] = np.ascontiguousarray(np.asarray(inp["fc2_w"], np.float32).T)
    for name in ("conv_b", "q_b", "k_b", "v_b", "bn1_g", "bn1_b",
                 "bn2_g", "bn2_b", "fc1_b", "fc2_b"):
        prep[name] = np.ascontiguousarray(
            np.asarray(inp[name], np.float32).reshape(-1, 1))
    return im2colT, prep


def kernel(**inputs):
    nc = build()
    im2colT, prep = host_prep(inputs)
    im3 = im2colT.reshape(KFEAT, B, S)
    in_maps = []
    for c in range(N_CORES):
        blk = im3[:, c * B_LOC:(c + 1) * B_LOC, :]
        pad = np.zeros((KFEAT, B_PAD - B_LOC, S), np.float32)
        im2_c = np.ascontiguousarray(
            np.concatenate([blk, pad], axis=1).reshape(KFEAT, POS_LOC))
        m = {"im2": im2_c}
        m.update(prep)
        in_maps.append(m)
    res = run_bass_kernel_spmd(nc, in_maps, list(range(N_CORES)))
    outs = [res.results[c]["out"] for c in range(N_CORES)]  # [2, 512] each
    return np.ascontiguousarray(
        np.concatenate([o.T for o in outs], axis=0)).astype(np.float32)


if __name__ == "__main__":
    import reference
    inp = reference.setup_inputs()
    out = kernel(**{k: np.asarray(v) for k, v in inp.items()})
    print(out.shape, out.dtype)


# revision 21
# speedup vs baseline: 74.8796x; 74.8796x over previous
"""Trainium2 Bass kernel for nn_CTNet (conv -> BN -> MHSA -> BN -> maxpool -> MLP).

Data-parallel over 8 NeuronCores: batch 4096 -> 512/core (padded to 516 =
43 attention rounds x 12 samples). Training-mode BatchNorm batch stats are
made global with two tiny AllReduces ([128,8] mean/E[x^2] payloads).

Layouts (per core):
  xf / q / k / av : [C(4x128 part), pos]   pos = sample*10 + s
  vT              : [pos(120-chunks), C]   (direct transposed projection)
  attention       : 12 samples/round; 4 PE column-strips x 3 samples each,
                    3-sample Gram blocks masked to the diagonal, softmax over
                    30 cols with -1e30 off-block mask, attn^T via PE
                    transpose into a 120x120 block-diagonal operand
  pooled          : [C, (p 8, b 516)] ; maxpool commutes with the (positive
                    scale) BN2 affine + ReLU, which are applied post-pool
  FC1             : weights host-permuted to [(p*512+c), h]; rhs = pooled
                    slices; out [h, b]; 2 PSUM waves of 8 h-chunks
"""
import sys
import functools
from contextlib import ExitStack

sys.path.insert(0, "/opt/trn_rl_repo")

import numpy as np
import concourse.bass as bass  # noqa: E402
import concourse.tile as tile  # noqa: E402
from concourse import bacc, mybir  # noqa: E402
from concourse.bass_utils import run_bass_kernel_spmd  # noqa: E402

N_CORES = 8
B = 4096
B_LOC = 512
B_PAD = 516
S = 10
C = 512
CH, H, W = 7, 4, 17
KH, KW = 4, 10
KFEAT = 280
POS_LOC = B_PAD * S        # 5160
POS_REAL = B_LOC * S       # 5120
HID = 2048
EPS = 1e-5
SB_SAMP = 36
N_SB = 15                  # 14 x 36 + 12 samples
NEG = -1e30

F32 = mybir.dt.float32
F32R = mybir.dt.float32r
AF = mybir.ActivationFunctionType
ALU = mybir.AluOpType
AX = mybir.AxisListType





def mask30_np():
    m = np.full((128, 30), NEG, np.float32)
    for c in range(4):
        for s in range(3):
            m[32 * c + 10 * s: 32 * c + 10 * s + 10, 10 * s: 10 * s + 10] = 0.0
    return m


def identb_np():
    return np.eye(128, dtype=np.float32)


def _emit(nc, tc, ctx, d):
    const = ctx.enter_context(tc.tile_pool(name="const", bufs=1))
    small = ctx.enter_context(tc.tile_pool(name="small", bufs=4))

    def load_const(dram, shape, name=None, dt=F32):
        t = const.tile(list(shape), dt, tag=name, name=name)
        if dram.dtype != dt:
            dram = dram.bitcast(dt)
        nc.sync.dma_start(t[:, :], dram)
        return t

    # ---- resident parameters ----
    wim_t = [load_const(d["wim"][k0:k1, :], [k1 - k0, 512], f"wim{i}", F32R)
             for i, (k0, k1) in enumerate(((0, 128), (128, 256), (256, 280)))]
    qT_t = [load_const(d["qT"][kc * 128:(kc + 1) * 128, :], [128, 512], f"qT{kc}", F32R)
            for kc in range(4)]
    kT_t = [load_const(d["kT"][kc * 128:(kc + 1) * 128, :], [128, 512], f"kT{kc}", F32R)
            for kc in range(4)]
    vwT_t = [load_const(d["vwT"][kc * 128:(kc + 1) * 128, :], [128, 512], f"vwT{kc}", F32R)
             for kc in range(4)]
    pos3_t = [load_const(d["pos3"][kc * 128:(kc + 1) * 128, :], [128, 30], f"pos3{kc}", F32R)
              for kc in range(4)]
    fc2T_t = [load_const(d["fc2T"][h8 * 128:(h8 + 1) * 128, :], [128, 2], f"fc2T{h8}", F32R)
              for h8 in range(16)]

    def vec4(name):
        return [load_const(d[name][mc * 128:(mc + 1) * 128, :], [128, 1],
                           f"{name}{mc}") for mc in range(4)]

    cb_t = vec4("conv_b")
    qb_t = vec4("q_b")
    kb_t = vec4("k_b")
    vb_t = vec4("v_b")
    bn1g_t = vec4("bn1_g")
    bn1b_t = vec4("bn1_b")
    bn2g_t = vec4("bn2_g")
    bn2b_t = vec4("bn2_b")
    fc1b_t = [load_const(d["fc1_b"][h8 * 128:(h8 + 1) * 128, :], [128, 1],
                         f"fc1b{h8}") for h8 in range(16)]
    fc2b_t = load_const(d["fc2_b"], [2, 1], "fc2b")
    mask_t = load_const(d["mask30"].ap(), [128, 30], "mask")
    id128_t = load_const(d["identb"].ap(), [128, 128], "identb", F32R)

    # ---- persistent state ----
    scores = const.tile([128, 43 * 30], F32, tag="scores", name="scores")
    bd = const.tile([128, 256], F32R, tag="bd", name="bd")
    nc.sync.dma_start(bd[:, :], d["zeros"].ap().bitcast(F32R))
    stats1 = [const.tile([128, 72], F32, tag=f"st1_{mc}", name=f"st1_{mc}") for mc in range(4)]
    stats2 = [const.tile([128, 264], F32, tag=f"st2_{mc}", name=f"st2_{mc}") for mc in range(4)]
    pooled = [const.tile([128, 8 * B_PAD], F32R, tag=f"pool{mc}", name=f"pool{mc}") for mc in range(4)]
    scale1 = [const.tile([128, 1], F32, tag=f"sc1_{mc}", name=f"sc1_{mc}") for mc in range(4)]
    shift1 = [const.tile([128, 1], F32, tag=f"sh1_{mc}", name=f"sh1_{mc}") for mc in range(4)]
    scale2 = [const.tile([128, 1], F32, tag=f"sc2_{mc}", name=f"sc2_{mc}") for mc in range(4)]
    shift2 = [const.tile([128, 1], F32, tag=f"sh2_{mc}", name=f"sh2_{mc}") for mc in range(4)]

    # ================= PASS 1: conv =================
    with (
        tc.tile_pool(name="im2p", bufs=6) as im2p,
        tc.tile_pool(name="cvps", bufs=4, space="PSUM") as cvps,
        tc.tile_pool(name="xfst", bufs=4) as xfst,
    ):
        for nb in range(12):
            n0 = nb * 430
            im2t = []
            for i, (k0, k1) in enumerate(((0, 128), (128, 256), (256, 280))):
                t = im2p.tile([128, 430], F32R, tag="im2t", name="im2t")
                nc.sync.dma_start(t[: k1 - k0, :], d["im2"][k0:k1, n0:n0 + 430])
                im2t.append(t)
            for mc in range(4):
                ps = cvps.tile([128, 430], F32, name="cvps")
                for i, (k0, k1) in enumerate(((0, 128), (128, 256), (256, 280))):
                    nc.tensor.matmul(
                        ps[:, :],
                        wim_t[i][: k1 - k0, mc * 128:(mc + 1) * 128],
                        im2t[i][: k1 - k0, :],
                        start=(i == 0), stop=(i == 2))
                xs = xfst.tile([128, 430], F32)
                nc.scalar.activation(xs[:, :], ps[:, :], AF.Identity, bias=cb_t[mc][:, :])
                w = 430 if nb < 11 else 390
                nc.vector.bn_stats(stats1[mc][:, nb * 6:(nb + 1) * 6], xs[:, :w])
                nc.sync.dma_start(
                    d["xf"][mc * 128:(mc + 1) * 128, n0:n0 + 430], xs[:, :])

    # ================= AllReduce 1 (BN1 stats) =================
    def bn_allreduce(statsbufs, cc_in, cc_out, g_t, b_t, scale_t, shift_t,
                     corr=None):
        # corr = (a, b, dead_t): true mean = a*mean_meas - b*dead,
        #        true E[x^2] = a*e2_meas - b*dead^2
        pay = small.tile([128, 8], F32, tag="pay", name="pay")
        eps_t = small.tile([128, 1], F32, tag="eps", name="eps")
        nc.vector.memset(eps_t[:, :], EPS)
        for mc in range(4):
            loc = small.tile([128, 2], F32, tag="bnloc", name="bnloc")
            nc.vector.bn_aggr(loc[:, :], statsbufs[mc][:, :])
            e2m = small.tile([128, 1], F32, tag="e2m", name="e2m")
            nc.vector.scalar_tensor_tensor(
                e2m[:, :], loc[:, 0:1], loc[:, 0:1],
                loc[:, 1:2], op0=ALU.mult, op1=ALU.add)
            if corr is None:
                nc.vector.tensor_copy(pay[:, 2 * mc:2 * mc + 1], loc[:, 0:1])
                nc.vector.tensor_copy(pay[:, 2 * mc + 1:2 * mc + 2], e2m[:, :])
            else:
                ca, cb, dead_t = corr
                db = small.tile([128, 1], F32, tag="db", name="db")
                nc.scalar.mul(db[:, :], dead_t[mc][:, :], cb)
                nc.vector.scalar_tensor_tensor(
                    pay[:, 2 * mc:2 * mc + 1], loc[:, 0:1], ca, db[:, :],
                    op0=ALU.mult, op1=ALU.subtract)
                d2b = small.tile([128, 1], F32, tag="d2b", name="d2b")
                nc.vector.tensor_mul(d2b[:, :], db[:, :], dead_t[mc][:, :])
                nc.vector.scalar_tensor_tensor(
                    pay[:, 2 * mc + 1:2 * mc + 2], e2m[:, :], ca, d2b[:, :],
                    op0=ALU.mult, op1=ALU.subtract)
        nc.sync.dma_start(cc_in.ap(), pay[:, :])
        nc.gpsimd.collective_compute(
            "AllReduce", ALU.add, replica_groups=[list(range(N_CORES))],
            ins=[cc_in.ap()], outs=[cc_out.ap()])
        ar = small.tile([128, 8], F32, tag="ar", name="ar")
        nc.sync.dma_start(ar[:, :], cc_out.ap())
        for mc in range(4):
            mg = small.tile([128, 1], F32, tag="mg", name="mg")
            e2 = small.tile([128, 1], F32, tag="e2", name="e2")
            nc.scalar.mul(mg[:, :], ar[:, 2 * mc:2 * mc + 1], 1.0 / N_CORES)
            nc.scalar.mul(e2[:, :], ar[:, 2 * mc + 1:2 * mc + 2], 1.0 / N_CORES)
            mg2 = small.tile([128, 1], F32, tag="mg2", name="mg2")
            nc.vector.tensor_mul(mg2[:, :], mg[:, :], mg[:, :])
            vg = small.tile([128, 1], F32, tag="vg", name="vg")
            nc.vector.tensor_sub(vg[:, :], e2[:, :], mg2[:, :])
            sq = small.tile([128, 1], F32, tag="sq", name="sq")
            nc.scalar.activation(sq[:, :], vg[:, :], AF.Sqrt, bias=eps_t[:, :])
            rstd = small.tile([128, 1], F32, tag="rstd", name="rstd")
            nc.vector.reciprocal(rstd[:, :], sq[:, :])
            nc.vector.tensor_mul(scale_t[mc][:, :], rstd[:, :], g_t[mc][:, :])
            ms = small.tile([128, 1], F32, tag="ms", name="ms")
            nc.vector.tensor_mul(ms[:, :], mg[:, :], scale_t[mc][:, :])
            nc.vector.tensor_sub(shift_t[mc][:, :], b_t[mc][:, :], ms[:, :])

    bn_allreduce(stats1, d["cc1i"], d["cc1o"], bn1g_t, bn1b_t, scale1, shift1)

    # ================= PASS 2: attention super-blocks =================
    with (
        tc.tile_pool(name="xfl", bufs=4) as xfl,
        tc.tile_pool(name="xfn", bufs=8) as xfnp,
        tc.tile_pool(name="qk", bufs=16) as qkp,
        tc.tile_pool(name="vt", bufs=6) as vtp,
        tc.tile_pool(name="att", bufs=4) as attp,
        tc.tile_pool(name="avs", bufs=8) as avsp,
        tc.tile_pool(name="ptmp", bufs=4) as ptmpp,
        tc.tile_pool(name="qkps", bufs=2, space="PSUM") as qkps,
        tc.tile_pool(name="vtps", bufs=2, space="PSUM") as vtps,
        tc.tile_pool(name="scps", bufs=1, space="PSUM") as scps,
        tc.tile_pool(name="trps", bufs=1, space="PSUM") as trps,
        tc.tile_pool(name="avps", bufs=2, space="PSUM") as avps,
    ):
        for sb in range(N_SB):
            nsamp = SB_SAMP if sb < 14 else 12
            L = nsamp * S
            n_r = nsamp // 12
            LG = n_r * 128          # gapped width: rounds are 4 strips x 32
            pos0 = sb * SB_SAMP * S
            glob0 = sb * SB_SAMP
            # load + BN1-normalize xf (dense DRAM -> gapped SBUF layout)
            xfn = []
            for mc in range(4):
                xr = xfl.tile([128, 3 * 128], F32, tag="xfl", name="xfl")
                nc.vector.memset(xr[:, :LG], 0.0)
                src = d["xf"][mc * 128:(mc + 1) * 128, pos0:pos0 + L].rearrange(
                    "c (r t k) -> c r t k", r=n_r, t=4)
                dst = xr[:, :LG].rearrange("c (r t k) -> c r t k",
                                           r=n_r, t=4, k=32)[:, :, :, 0:30]
                nc.sync.dma_start(dst, src)
                xn = xfnp.tile([128, 3 * 128], F32R, tag="xfn", name="xfn")
                nc.scalar.activation(xn[:, :LG], xr[:, :LG], AF.Relu,
                                     bias=shift1[mc][:, :], scale=scale1[mc][:, :])
                xfn.append(xn)
            # q, k projections [C, LG]
            q_t, k_t = [], []
            for wT, bias_t, outlist in ((qT_t, qb_t, q_t), (kT_t, kb_t, k_t)):
                for mc in range(4):
                    ps = qkps.tile([128, 3 * 128], F32, name="qkps")
                    for kc in range(4):
                        nc.tensor.matmul(
                            ps[:, :LG],
                            wT[kc][:, mc * 128:(mc + 1) * 128],
                            xfn[kc][:, :LG],
                            start=(kc == 0), stop=(kc == 3))
                    o = qkp.tile([128, 3 * 128], F32R, tag="qk", name="qk")
                    nc.scalar.activation(o[:, :LG], ps[:, :LG], AF.Identity,
                                         bias=bias_t[mc][:, :])
                    outlist.append(o)
            # vT [128-chunk (gapped pos), C]
            vt_t = []
            for pc in range(n_r):
                ps = vtps.tile([128, 512], F32, name="vtps")
                for kc in range(4):
                    nc.tensor.matmul(
                        ps[:, :],
                        xfn[kc][:, pc * 128:(pc + 1) * 128],
                        vwT_t[kc][:, :],
                        start=(kc == 0), stop=(kc == 3))
                vt = vtp.tile([128, 512], F32R, tag="vt", name="vt")
                nc.vector.tensor_copy(vt[:, :], ps[:, :])
                vt_t.append(vt)
            # scores per round
            for r in range(n_r):
                g = sb * 3 + r
                ps = scps.tile([128, 30], F32, name="scps")
                for strip in range(4):
                    sl0 = r * 128 + strip * 32
                    mm = 0
                    for kc in range(4):
                        nc.tensor.matmul(
                            ps[32 * strip:32 * strip + 30, :],
                            q_t[kc][:, sl0:sl0 + 30].bitcast(F32),
                            k_t[kc][:, sl0:sl0 + 30].bitcast(F32),
                            start=(mm == 0), stop=False,
                            tile_position=(0, 32 * strip))
                        mm += 1
                    for kc in range(4):
                        nc.tensor.matmul(
                            ps[32 * strip:32 * strip + 30, :],
                            pos3_t[kc][:, :].bitcast(F32),
                            q_t[kc][:, sl0:sl0 + 30].bitcast(F32),
                            start=False, stop=(mm == 7),
                            tile_position=(0, 32 * strip))
                        mm += 1
                nc.vector.scalar_tensor_tensor(
                    scores[:, g * 30:(g + 1) * 30], ps[:, :], 1.0, mask_t[:, :],
                    op0=ALU.mult, op1=ALU.add)
            # softmax over this SB's rounds
            g0 = sb * 3
            sc3 = scores[:, g0 * 30:(g0 + n_r) * 30].rearrange(
                "p (r j) -> p r j", r=n_r)
            mx = small.tile([128, 4], F32, tag="mx", name="mx")
            nc.vector.tensor_reduce(mx[:, :n_r], sc3, axis=AX.X, op=ALU.max)
            att = attp.tile([128, 90], F32, tag="att", name="att")
            att3 = att[:, :n_r * 30].rearrange("p (r j) -> p r j", r=n_r)
            mx3 = mx[:, :n_r].unsqueeze(2).broadcast_to([128, n_r, 30])
            nc.vector.tensor_tensor(att3, sc3, mx3, op=ALU.subtract)
            nc.scalar.activation(att[:, :n_r * 30], att[:, :n_r * 30], AF.Exp)
            sm = small.tile([128, 4], F32, tag="sm", name="sm")
            nc.vector.tensor_reduce(sm[:, :n_r], att3, axis=AX.X, op=ALU.add)
            rs = small.tile([128, 4], F32, tag="rs", name="rs")
            nc.vector.reciprocal(rs[:, :n_r], sm[:, :n_r])
            attn = attp.tile([128, 90], F32R, tag="attn", name="attn")
            attn3 = attn[:, :n_r * 30].rearrange("p (r j) -> p r j", r=n_r)
            rs3 = rs[:, :n_r].unsqueeze(2).broadcast_to([128, n_r, 30])
            nc.vector.tensor_tensor(attn3, att3, rs3, op=ALU.mult)
            # av per round
            av_t = []
            for mc in range(4):
                av_t.append(avsp.tile([128, 3 * 128], F32, tag="avs", name="avs"))
            for r in range(n_r):
                g = sb * 3 + r
                half = g % 2
                pst = trps.tile([30, 128], F32R, name="trps")
                nc.tensor.transpose(
                    pst[:, :],
                    attn[:, r * 30:(r + 1) * 30],
                    id128_t[:, :])
                for strip in range(4):
                    nc.scalar.copy(
                        bd[32 * strip:32 * strip + 30,
                           half * 128 + 32 * strip: half * 128 + 32 * strip + 30],
                        pst[0:30, 32 * strip:32 * strip + 30])
                for mc in range(4):
                    ap = avps.tile([128, 128], F32, name="avpst")
                    nc.tensor.matmul(
                        ap[:, :],
                        vt_t[r][:, mc * 128:(mc + 1) * 128],
                        bd[:, half * 128: half * 128 + 128],
                        start=True, stop=True)
                    nc.scalar.activation(
                        av_t[mc][:, r * 128:(r + 1) * 128], ap[:, :],
                        AF.Identity, bias=vb_t[mc][:, :])
            # BN2 stats: whole 128-col round blocks; the 8 dead lanes per
            # block hold exactly v_b (bd zeros kill everything else) and are
            # corrected analytically in bn_allreduce. Round 42: only strips
            # 0,1 (60 real + 4 dead) and the 20 real cols of strip 2.
            for r in range(n_r):
                g = sb * 3 + r
                for mc in range(4):
                    avr = av_t[mc][:, r * 128:(r + 1) * 128]
                    if g < 42:
                        nc.vector.bn_stats(
                            stats2[mc][:, g * 6:(g + 1) * 6], avr[:, :])
                    else:
                        nc.vector.bn_stats(
                            stats2[mc][:, g * 6:(g + 1) * 6], avr[:, 0:64])
                        nc.vector.bn_stats(
                            stats2[mc][:, (g + 1) * 6:(g + 2) * 6],
                            avr[:, 64:84])
            # maxpool (1,3) -> pooled [C, (p 8, b 516)]
            for mc in range(4):
                po4 = pooled[mc].rearrange("c (p g u) -> c g u p",
                                           p=8, g=43, u=12)
                for strip in range(4):
                    av5 = av_t[mc][:, :LG].rearrange(
                        "c (r t k) -> c r t k", r=n_r, t=4, k=32)[
                            :, :, strip, 0:30].rearrange(
                            "c r (s i) -> c r s i", s=3)
                    pt = ptmpp.tile([128, 3 * 24], F32, tag="ptmp", name="ptmp")
                    pt4 = pt[:, :n_r * 24].rearrange(
                        "c (r s p) -> c r s p", r=n_r, s=3)
                    nc.vector.tensor_tensor(pt4, av5[:, :, :, 0:8],
                                            av5[:, :, :, 1:9], op=ALU.max)
                    nc.vector.tensor_tensor(
                        po4[:, sb * 3:sb * 3 + n_r,
                            3 * strip:3 * strip + 3, :],
                        pt4, av5[:, :, :, 2:10], op=ALU.max)

    # ================= AllReduce 2 (BN2 stats) =================
    n_meas = 42 * 128 + 64 + 20
    bn_allreduce(stats2, d["cc2i"], d["cc2o"], bn2g_t, bn2b_t, scale2, shift2,
                 corr=(n_meas / POS_REAL, (n_meas - POS_REAL) / POS_REAL, vb_t))

    # ================= PASS 3: BN2 affine + FC head =================
    for mc in range(4):
        nc.scalar.activation(pooled[mc][:, :], pooled[mc][:, :], AF.Relu,
                             bias=shift2[mc][:, :], scale=scale2[mc][:, :])
    with tc.tile_pool(name="h1p", bufs=1) as h1p:
      h1_t = [h1p.tile([128, B_LOC], F32R, tag=f"h1_{h8}", name=f"h1_{h8}")
              for h8 in range(16)]
      with (
        tc.tile_pool(name="fc1w", bufs=3) as fc1wp,
        tc.tile_pool(name="fc1ps", bufs=8, space="PSUM") as fc1ps,
      ):
        for wave in range(2):
            hps = [fc1ps.tile([128, 512], F32, name="fc1pst") for _ in range(8)]
            for fk in range(32):
                p, mc = divmod(fk, 4)
                fkt = fc1wp.tile([128, 2048], F32R, tag="fc1w", name="fc1w")
                nc.sync.dma_start(fkt[:, :], d["fc1p"][fk * 128:(fk + 1) * 128, :])
                for i in range(8):
                    h8 = wave * 8 + i
                    nc.tensor.matmul(
                        hps[i][:, :],
                        fkt[:, h8 * 128:(h8 + 1) * 128],
                        pooled[mc][:, p * B_PAD: p * B_PAD + B_LOC],
                        start=(fk == 0), stop=(fk == 31))
            for i in range(8):
                h8 = wave * 8 + i
                nc.scalar.activation(h1_t[h8][:, :], hps[i][:, :], AF.Relu,
                                     bias=fc1b_t[h8][:, :])
      with (
          tc.tile_pool(name="fc2ps", bufs=1, space="PSUM") as fc2ps,
          tc.tile_pool(name="outp", bufs=1) as outp,
      ):
        ps = fc2ps.tile([2, 512], F32, name="fc2pst")
        for h8 in range(16):
            nc.tensor.matmul(ps[:, :], fc2T_t[h8][:, :], h1_t[h8][:, :],
                             start=(h8 == 0), stop=(h8 == 15))
        out_sb = outp.tile([2, 512], F32, tag="outsb", name="outsb")
        nc.scalar.activation(out_sb[:, :], ps[:, :], AF.Sigmoid,
                             bias=fc2b_t[:, :])
        nc.sync.dma_start(d["out"].ap(), out_sb[:, :])


@functools.lru_cache(maxsize=1)
def build():
    nc = bacc.Bacc("TRN2", target_bir_lowering=False, debug=False,
                   num_devices=N_CORES)
    d = {}
    d["im2"] = nc.dram_tensor("im2", [KFEAT, POS_LOC], F32R,
                              kind="ExternalInput").ap()
    for name, shape in (
        ("wim", [KFEAT, 512]), ("qT", [512, 512]), ("kT", [512, 512]),
        ("vwT", [512, 512]), ("pos3", [512, 30]), ("conv_b", [512, 1]),
        ("q_b", [512, 1]), ("k_b", [512, 1]), ("v_b", [512, 1]),
        ("bn1_g", [512, 1]), ("bn1_b", [512, 1]), ("bn2_g", [512, 1]),
        ("bn2_b", [512, 1]), ("fc1p", [C * 8, HID]), ("fc1_b", [HID, 1]),
        ("fc2T", [HID, 2]), ("fc2_b", [2, 1]),
    ):
        dt = F32R if name in ("wim", "qT", "kT", "vwT", "pos3", "fc1p",
                              "fc2T") else F32
        d[name] = nc.dram_tensor(name, shape, dt, kind="ExternalInput").ap()
    d["out"] = nc.dram_tensor("out", [2, B_LOC], F32, kind="ExternalOutput")
    d["xf"] = nc.dram_tensor("xf", [C, POS_LOC], F32).ap()
    d["cc1i"] = nc.dram_tensor("cc1i", [128, 8], F32)
    d["cc1o"] = nc.dram_tensor("cc1o", [128, 8], F32, addr_space="Shared")
    d["cc2i"] = nc.dram_tensor("cc2i", [128, 8], F32)
    d["cc2o"] = nc.dram_tensor("cc2o", [128, 8], F32, addr_space="Shared")
    d["mask30"] = nc.inline_tensor(mask30_np(), name="mask30")
    d["identb"] = nc.inline_tensor(identb_np(), name="identb")
    d["zeros"] = nc.inline_tensor(np.zeros((128, 256), np.float32), name="zeros")

    with tile.TileContext(nc) as tc:
        with ExitStack() as ctx:
            _emit(nc, tc, ctx, d)
    nc.compile()
    return nc


def host_prep(inp):
    x = np.asarray(inp["x"], np.float32)
    x_pad = np.zeros((B, CH, H, W + 2), np.float32)
    x_pad[:, :, :, 1:-1] = x
    sw = np.lib.stride_tricks.sliding_window_view(x_pad, KW, axis=3)
    im2colT = np.ascontiguousarray(
        sw.transpose(1, 2, 4, 0, 3).reshape(KFEAT, B * S))
    prep = {}
    prep["wim"] = np.ascontiguousarray(
        np.asarray(inp["conv_w"], np.float32).reshape(C, KFEAT).T)
    prep["qT"] = np.ascontiguousarray(np.asarray(inp["q_w"], np.float32).T)
    prep["kT"] = np.ascontiguousarray(np.asarray(inp["k_w"], np.float32).T)
    prep["vwT"] = np.ascontiguousarray(np.asarray(inp["v_w"], np.float32).T)
    pos = (np.asarray(inp["rel_h"], np.float32)
           + np.asarray(inp["rel_w"], np.float32)).reshape(C, S)
    prep["pos3"] = np.ascontiguousarray(np.concatenate([pos] * 3, axis=1))
    prep["fc1p"] = np.ascontiguousarray(
        np.asarray(inp["fc1_w"], np.float32).reshape(HID, C, 8)
        .transpose(2, 1, 0).reshape(C * 8, HID))
    prep["fc2T"] = np.ascontiguousarray(np.asarray(inp["fc2_w"], np.float32).T)
    for name in ("conv_b", "q_b", "k_b", "v_b", "bn1_g", "bn1_b",
                 "bn2_g", "bn2_b", "fc1_b", "fc2_b"):
        prep[name] = np.ascontiguousarray(
            np.asarray(inp[name], np.float32).reshape(-1, 1))
    return im2colT, prep


@functools.lru_cache(maxsize=1)
def _runner():
    """Jit-compile the SPMD executable once; reuse across kernel() calls."""
    import jax
    from jax.sharding import Mesh, PartitionSpec
    from jax.experimental.shard_map import shard_map
    from concourse import bass2jax, mybir as _mybir

    nc = build()
    bass2jax.install_neuronx_cc_hook()
    pname = nc.partition_id_tensor.name if nc.partition_id_tensor else None
    in_names, out_names, out_avals, zero_outs = [], [], [], []
    for alloc in nc.m.functions[0].allocations:
        if not isinstance(alloc, _mybir.MemoryLocationSet):
            continue
        name = alloc.memorylocations[0].name
        if alloc.kind == "ExternalInput":
            if name == pname:
                continue
            in_names.append(name)
        elif alloc.kind == "ExternalOutput":
            shape = tuple(alloc.tensor_shape)
            dtype = _mybir.dt.np(alloc.dtype)
            out_names.append(name)
            out_avals.append(jax.core.ShapedArray(shape, dtype))
            zero_outs.append(np.zeros(shape, dtype))
    n_params = len(in_names)
    n_outs = len(out_names)
    all_names = in_names + out_names
    if pname is not None:
        all_names = all_names + [pname]

    def _body(*args):
        operands = list(args)
        if pname is not None:
            operands.append(bass2jax.partition_id_tensor())
        outs = bass2jax._bass_exec_p.bind(
            *operands,
            out_avals=tuple(out_avals),
            in_names=tuple(all_names),
            out_names=tuple(out_names),
            lowering_input_output_aliases=(),
            sim_require_finite=True,
            sim_require_nnan=True,
            nc=nc,
        )
        return tuple(outs)

    devices = jax.devices()[:N_CORES]
    mesh = Mesh(np.asarray(devices), ("core",))
    donate = tuple(range(n_params, n_params + n_outs))
    fn = jax.jit(
        shard_map(_body, mesh=mesh,
                  in_specs=(PartitionSpec("core"),) * (n_params + n_outs),
                  out_specs=(PartitionSpec("core"),) * n_outs,
                  check_rep=False),
        donate_argnums=donate, keep_unused=True)
    return fn, in_names, out_names, out_avals, zero_outs


def make_in_maps(inputs):
    im2colT, prep = host_prep(inputs)
    im3 = im2colT.reshape(KFEAT, B, S)
    in_maps = []
    for c in range(N_CORES):
        blk = im3[:, c * B_LOC:(c + 1) * B_LOC, :]
        pad = np.zeros((KFEAT, B_PAD - B_LOC, S), np.float32)
        im2_c = np.ascontiguousarray(
            np.concatenate([blk, pad], axis=1).reshape(KFEAT, POS_LOC))
        m = {"im2": im2_c}
        m.update(prep)
        in_maps.append(m)
    return in_maps


def run_spmd(in_maps):
    fn, in_names, out_names, out_avals, zero_outs = _runner()
    concat_in = [
        np.concatenate([in_maps[c][nm] for c in range(N_CORES)], axis=0)
        for nm in in_names]
    concat_zeros = [
        np.zeros((N_CORES * z.shape[0], *z.shape[1:]), z.dtype)
        for z in zero_outs]
    out_arrs = fn(*concat_in, *concat_zeros)
    return [
        {nm: np.asarray(out_arrs[i]).reshape(N_CORES, *out_avals[i].shape)[c]
         for i, nm in enumerate(out_names)}
        for c in range(N_CORES)]


def kernel(**inputs):
    results = run_spmd(make_in_maps(inputs))
    outs = [results[c]["out"] for c in range(N_CORES)]  # [2, 512] each
    return np.ascontiguousarray(
        np.concatenate([o.T for o in outs], axis=0)).astype(np.float32)


if __name__ == "__main__":
    import reference
    inp = reference.setup_inputs()
    out = kernel(**{k: np.asarray(v) for k, v in inp.items()})
    print(out.shape, out.dtype)
